# revision 5
# baseline (speedup 1.0000x reference)
"""CAM (channel attention) module kernel for Trainium2, SPMD over 8 NeuronCores.

Reference computation (per batch b):
    q = x[b].reshape(C, N)                  # C=64, N=H*W=65536
    energy = q @ q.T                        # [C, C]
    att = softmax(rowmax(energy) - energy)  # == softmax(-energy) rows
    out[b] = gamma * (att @ q) + x[b]

Sharding: data-parallel over batch, 2 batches per core, no cross-core comm.

Per-core design (v3):

  Layout: q2 [128, 32768] fp32 where partition p = h*64 + c (h = n-half,
  c = channel), streamed in [128, 4096] tiles (two [64, 4096] DMAs each;
  16 KiB contiguous rows amortize the ~300ns/descriptor SDMA overhead so
  a single HWDGE queue sustains ~300 GB/s instead of ~200 at 8 KiB rows).

  Split: hi = bf16(q) (DVE 2x copy), lo = bf16(q - hi) (fp32 TT, split
  DVE/GpSimd). hi+lo reconstructs q to ~2^-18; hi-only energy fails
  (rel 0.16) but hi/lo energy + hi-only phase-2 passes at ~3e-3.

  Phase 1 (energy): PE-transpose [128, 128] bf16 blocks of hi/lo, stage
  PPG pairs [Thi|Tlo] per PSUM group, copy to SBUF (ACT/DVE/GpSimd round
  robin), then bf16 grams accumulate acc[:,0:128] += T^T@Thi (diag-block
  energies) and acc[:,128:256] += T^T@Tlo (hi/lo cross term).
  E = (G00+G11) + (X00+X11) + (X00+X11)^T via matmul against the stacked
  double identity.

  Softmax: att = exp(rmin(E) - E) / rowsum (shift-invariant, matches the
  reference). S = blockdiag(M^T, M^T), M = gamma*att + I, cast bf16.

  Phase 2: out_slab = S_hi @ hi_slab (single bf16 matmul; the identity
  carries the hi residual; dropping the lo terms costs ~2e-3 rel).

  Schedule: reads on the Sync HWDGE ring, writes on the Scalar HWDGE
  ring (independent rings overlap read+write streams). Batch 1's
  load/split/phase-1 is interleaved tile-by-tile with batch 0's
  phase-2/stores so the write stream starts ~55us in and the PE never
  sits behind a write-paced head-of-line stall.
"""

import os

import numpy as np

import concourse.bass as bass
import concourse.tile as tile
from concourse import bacc, mybir

# Problem constants (hardcoded per harness contract).
B, C, H, W = 16, 64, 256, 256
N = H * W  # 65536
NCORES = 8
BPC = B // NCORES  # batches per core

# Tunables.
TILE_F = 4096  # free width of a q2 tile ([64, TILE_F] fp32 DMA rows)
CHUNK = 128  # n'-block width (covers both halves per transpose)
PPG = 8  # transpose pairs per PSUM staging group
SLAB = 512  # phase-2 moving width (one PSUM bank of fp32)


def build_cam_program(n=N, bpc=BPC, tile_f=TILE_F):
    """Build the single-core Bass program (same program runs on all cores)."""
    half = n // 2
    ntiles = half // tile_f
    fp32 = mybir.dt.float32
    bf16 = mybir.dt.bfloat16

    nc = bacc.Bacc("TRN2", target_bir_lowering=False, debug=False)
    x = nc.dram_tensor("x", [bpc, C, n], fp32, kind="ExternalInput").ap()
    gamma = nc.dram_tensor("gamma", [1], fp32, kind="ExternalInput").ap()
    # ident: [128, 64] stacked double identity (fp32) for half-sum matmuls.
    ident = nc.dram_tensor("ident", [128, 64], fp32, kind="ExternalInput").ap()
    # identb: [128, 128] identity (bf16) as moving operand of bf16 transposes.
    identb = nc.dram_tensor("identb", [128, 128], bf16, kind="ExternalInput").ap()
    out = nc.dram_tensor("out", [bpc, C, n], fp32, kind="ExternalOutput").ap()

    blocks_per_tile = tile_f // CHUNK
    groups_per_tile = blocks_per_tile // PPG
    slabs_per_tile = tile_f // SLAB
    nblocks = ntiles * blocks_per_tile  # per batch

    with tile.TileContext(nc) as tc:
        with (
            tc.tile_pool(name="qpool", bufs=3) as qpool,
            tc.tile_pool(name="hipool", bufs=ntiles + 3) as hipool,
            tc.tile_pool(name="lopool", bufs=3) as lopool,
            tc.tile_pool(name="tpool", bufs=3) as tpool,
            tc.tile_pool(name="opool", bufs=2) as opool,
            tc.tile_pool(name="spool", bufs=1) as spool,
            tc.tile_pool(name="single", bufs=1) as single,
            tc.tile_pool(name="tps", bufs=2, space="PSUM") as tps_pool,
            tc.tile_pool(name="eps", bufs=1, space="PSUM") as eps_pool,
            tc.tile_pool(name="ops", bufs=3, space="PSUM") as ops_pool,
        ):
            # Constants ride the Scalar (qAct) HWDGE ring, which is idle at
            # start; x loads start immediately on the Sync (qSP) ring.
            ident_sb = single.tile([128, 64], fp32)
            nc.scalar.dma_start(out=ident_sb, in_=ident)
            identb_sb = single.tile([128, 128], bf16)
            nc.scalar.dma_start(out=identb_sb, in_=identb)
            gamma_sb = single.tile([128, 1], fp32)
            nc.scalar.dma_start(out=gamma_sb, in_=gamma.to_broadcast((128, 1)))

            # Warmup transpose: absorbs the identb-DMA wait on PE so real
            # transposes carry a single wait (LDWEIGHTS allows one).
            warm = ops_pool.tile([128, 128], bf16, tag="ops", name="warm")
            nc.tensor.transpose(warm, identb_sb, identb_sb)

            hitiles = {}  # (b, t) -> hi tile

            def load_split(b, t):
                """DMA one [128, tile_f] fp32 tile and hi/lo split it."""
                qt = qpool.tile([128, tile_f], fp32)
                nc.sync.dma_start(
                    out=qt[0:64, :], in_=x[b, :, t * tile_f : (t + 1) * tile_f]
                )
                nc.sync.dma_start(
                    out=qt[64:128, :],
                    in_=x[b, :, half + t * tile_f : half + (t + 1) * tile_f],
                )
                hi = hipool.tile([128, tile_f], bf16)
                nc.vector.tensor_copy(out=hi, in_=qt)  # DVE 2x fp32 copy-cast
                lo = lopool.tile([128, tile_f], bf16)
                # GpSimd owns the subtract (it cannot touch PSUM, so the
                # PSUM-read copies stay on DVE/ACT); split halves so the
                # first transposes start before the whole tile is split.
                hw = tile_f // 2
                nc.gpsimd.tensor_tensor(
                    out=lo[:, 0:hw],
                    in0=qt[:, 0:hw],
                    in1=hi[:, 0:hw],
                    op=mybir.AluOpType.subtract,
                )
                nc.gpsimd.tensor_tensor(
                    out=lo[:, hw:],
                    in0=qt[:, hw:],
                    in1=hi[:, hw:],
                    op=mybir.AluOpType.subtract,
                )
                hitiles[(b, t)] = hi
                return lo

            def phase1_tile(b, t, lo, acc, gcnt):
                """Transpose + gram one tile into the batch accumulator."""
                hi = hitiles[(b, t)]
                stage_eng = [nc.scalar, nc.vector]
                for g in range(groups_per_tile):
                    tps = tps_pool.tile([128, PPG * 256], bf16, tag="tps")
                    for i in range(PPG):
                        c = (g * PPG + i) * CHUNK
                        nc.tensor.transpose(
                            tps[:, i * 256 : i * 256 + 128],
                            hi[:, c : c + CHUNK],
                            identb_sb,
                        )
                        nc.tensor.transpose(
                            tps[:, i * 256 + 128 : (i + 1) * 256],
                            lo[:, c : c + CHUNK],
                            identb_sb,
                        )
                    tsb = tpool.tile([128, PPG * 256], bf16, tag="tsb")
                    eng = stage_eng[g % len(stage_eng)]
                    if eng is nc.scalar:
                        eng.copy(out=tsb, in_=tps)
                    else:
                        eng.tensor_copy(out=tsb, in_=tps)
                    for i in range(PPG):
                        nc.tensor.matmul(
                            acc,
                            lhsT=tsb[:, i * 256 : i * 256 + 128],
                            rhs=tsb[:, i * 256 : (i + 1) * 256],
                            start=(gcnt == 0),
                            stop=(gcnt == nblocks - 1),
                        )
                        gcnt += 1
                return gcnt

            def softmax_build_s(acc):
                """Combine energy halves, softmax, build the phase-2 stationary."""
                esb = spool.tile([128, 128], fp32)
                nc.vector.tensor_copy(out=esb[0:64, 0:64], in_=acc[0:64, 0:64])
                nc.vector.tensor_copy(
                    out=esb[64:128, 0:64], in_=acc[64:128, 64:128]
                )
                nc.vector.tensor_copy(
                    out=esb[0:64, 64:128], in_=acc[0:64, 128:192]
                )
                nc.vector.tensor_copy(
                    out=esb[64:128, 64:128], in_=acc[64:128, 192:256]
                )
                msum = ops_pool.tile([64, 128], fp32, tag="ops", name="msum")
                nc.tensor.matmul(
                    msum[:, 0:64],
                    lhsT=ident_sb,
                    rhs=esb[:, 0:64],
                    start=True,
                    stop=True,
                )
                nc.tensor.matmul(
                    msum[:, 64:128],
                    lhsT=ident_sb,
                    rhs=esb[:, 64:128],
                    start=True,
                    stop=True,
                )
                msb = spool.tile([64, 128], fp32)
                nc.vector.tensor_copy(out=msb, in_=msum)
                efull = spool.tile([64, 64], fp32)
                # E = E_hh + Xs + Xs^T
                xt = ops_pool.tile([64, 64], fp32, tag="ops", name="xt")
                nc.tensor.transpose(xt, msb[:, 64:128], ident_sb[0:64, :])
                nc.vector.tensor_add(efull, msb[:, 0:64], msb[:, 64:128])
                nc.vector.tensor_add(efull, efull, xt)

                # att = exp(rmin - E) / rowsum  (== softmax(rowmax(E)-E) rows)
                rmin = spool.tile([64, 1], fp32)
                nc.vector.tensor_reduce(
                    rmin, efull, axis=mybir.AxisListType.X, op=mybir.AluOpType.min
                )
                e2 = spool.tile([64, 128], fp32)
                nc.scalar.activation(
                    e2[:, 0:64],
                    efull,
                    mybir.ActivationFunctionType.Exp,
                    bias=rmin,
                    scale=-1.0,
                )
                ssum = spool.tile([64, 1], fp32)
                nc.vector.reduce_sum(ssum, e2[:, 0:64], axis=mybir.AxisListType.X)
                rsum = spool.tile([64, 1], fp32)
                nc.vector.reciprocal(rsum, ssum)
                att2 = spool.tile([64, 128], fp32)
                nc.vector.tensor_scalar_mul(att2[:, 0:64], e2[:, 0:64], rsum)
                nc.vector.tensor_copy(out=att2[:, 64:128], in_=att2[:, 0:64])
                # attT = [att^T ; att^T]
                atps = ops_pool.tile([128, 64], fp32, tag="ops", name="atps")
                nc.tensor.transpose(atps, att2, ident_sb[0:64, :])
                # S = blockdiag(M^T, M^T), M = gamma*att + I, cast bf16. The
                # identity carries the hi residual through phase 2.
                ssb = spool.tile([128, 128], fp32)
                nc.vector.memset(ssb, 0.0)
                nc.vector.tensor_scalar_mul(
                    ssb[0:64, 0:64], atps[0:64, :], gamma_sb[0:64]
                )
                nc.vector.tensor_scalar_mul(
                    ssb[64:128, 64:128], atps[64:128, :], gamma_sb[64:128]
                )
                nc.vector.tensor_add(
                    ssb[0:64, 0:64], ssb[0:64, 0:64], ident_sb[0:64, :]
                )
                nc.vector.tensor_add(
                    ssb[64:128, 64:128], ssb[64:128, 64:128], ident_sb[64:128, :]
                )
                s_hi = spool.tile([128, 128], bf16)
                nc.vector.tensor_copy(out=s_hi, in_=ssb)
                return s_hi

            def phase2_tile(b, t, s_hi):
                """out tile = S_hi @ hi tile, copy out, store (qAct ring)."""
                hi = hitiles.pop((b, t))
                osb = opool.tile([128, tile_f], fp32)
                copy_eng = [nc.scalar, nc.vector]
                for s in range(slabs_per_tile):
                    sl = slice(s * SLAB, (s + 1) * SLAB)
                    ops = ops_pool.tile([128, SLAB], fp32)
                    nc.tensor.matmul(
                        ops, lhsT=s_hi, rhs=hi[:, sl], start=True, stop=True
                    )
                    eng = copy_eng[s % len(copy_eng)]
                    if eng is nc.scalar:
                        eng.copy(out=osb[:, sl], in_=ops)
                    else:
                        eng.tensor_copy(out=osb[:, sl], in_=ops)
                nc.scalar.dma_start(
                    out=out[b, :, t * tile_f : (t + 1) * tile_f],
                    in_=osb[0:64, :],
                )
                nc.scalar.dma_start(
                    out=out[b, :, half + t * tile_f : half + (t + 1) * tile_f],
                    in_=osb[64:128, :],
                )

            # ---- Batch 0: load + phase 1 ----
            acc0 = eps_pool.tile([128, 256], fp32, tag="gacc")
            gcnt = 0
            lotiles0 = [load_split(0, t) for t in range(ntiles)]
            for t in range(ntiles):
                gcnt = phase1_tile(0, t, lotiles0[t], acc0, gcnt)
            s_hi0 = softmax_build_s(acc0)

            # ---- Mixed: batch 1 load/phase-1 interleaved with batch 0
            # phase-2/stores (keeps both HWDGE rings and the PE dense) ----
            acc1 = eps_pool.tile([128, 256], fp32, tag="gacc")
            gcnt = 0
            for t in range(ntiles):
                lo = load_split(1, t)
                gcnt = phase1_tile(1, t, lo, acc1, gcnt)
                phase2_tile(0, t, s_hi0)
            s_hi1 = softmax_build_s(acc1)

            # ---- Tail: batch 1 phase 2 ----
            for t in range(ntiles):
                phase2_tile(1, t, s_hi1)

    if not nc.is_finalized():
        nc.finalize()
    return nc


def _make_ident():
    ident = np.zeros((128, 64), np.float32)
    ident[np.arange(64), np.arange(64)] = 1.0
    ident[64 + np.arange(64), np.arange(64)] = 1.0
    return ident


def _make_identb():
    import ml_dtypes

    return np.eye(128, dtype=ml_dtypes.bfloat16)


def _setup_trace_hook():
    """Register the axon NTFF profiling hook (the image's antenv lacks the
    axon_hooks shim module; rebuild it and wire it to libaxon_pjrt.so)."""
    import sys
    import types

    import antenv

    if "antenv.axon_hooks" not in sys.modules:
        mod = types.ModuleType("antenv.axon_hooks")
        mod._hook = None

        def set_axon_ntff_profile_hook(h):
            mod._hook = h

        def get_axon_ntff_profile_hook():
            return mod._hook

        mod.set_axon_ntff_profile_hook = set_axon_ntff_profile_hook
        mod.get_axon_ntff_profile_hook = get_axon_ntff_profile_hook
        sys.modules["antenv.axon_hooks"] = mod
        antenv.axon_hooks = mod

    hooks = sys.modules["antenv.axon_hooks"]
    if hooks.get_axon_ntff_profile_hook() is None:
        from trn_agent_boot.trn_boot import _ntff_profile_via_ctypes

        hooks.set_axon_ntff_profile_hook(
            _ntff_profile_via_ctypes("/opt/axon/libaxon_pjrt.so")
        )

    # No S3 in this container: keep profile artifacts local.
    import concourse.bass_utils as bu

    bu.upload_artifacts = lambda tmpdir: tmpdir


def run(x, gamma, trace=False, tmpdir=None):
    """Run the SPMD kernel on 8 cores. Returns (out, exec_time_ns_or_None)."""
    from concourse.bass_utils import run_bass_kernel_spmd

    if trace:
        try:
            _setup_trace_hook()
        except Exception as e:  # tracing is best-effort; execution still works
            print("trace setup failed:", e)

    x = np.ascontiguousarray(np.asarray(x, dtype=np.float32))
    gamma = np.ascontiguousarray(np.asarray(gamma, dtype=np.float32))
    assert x.shape == (B, C, H, W), x.shape

    nc = build_cam_program()
    ident = _make_ident()
    identb = _make_identb()
    xr = x.reshape(NCORES, BPC, C, N)
    in_maps = [
        {
            "x": np.ascontiguousarray(xr[i]),
            "gamma": gamma,
            "ident": ident,
            "identb": identb,
        }
        for i in range(NCORES)
    ]
    res = run_bass_kernel_spmd(
        nc, in_maps, core_ids=list(range(NCORES)), trace=trace, tmpdir=tmpdir
    )
    outs = np.stack([np.asarray(res.results[i]["out"]) for i in range(NCORES)])
    y = outs.reshape(B, C, H, W).astype(np.float32)
    return y, res.exec_time_ns


def kernel(x, gamma):
    y, _ = run(x, gamma)
    return y


# revision 11
# speedup vs baseline: 1.0952x; 1.0952x over previous
"""CAM (channel attention) module kernel for Trainium2, SPMD over 8 NeuronCores.

Reference computation (per batch b):
    q = x[b].reshape(C, N)                  # C=64, N=H*W=65536
    energy = q @ q.T                        # [C, C]
    att = softmax(rowmax(energy) - energy)  # == softmax(-energy) rows
    out[b] = gamma * (att @ q) + x[b]

Sharding: data-parallel over batch, 2 batches per core, no cross-core comm.

Per-core design (v3):

  Layout: q2 [128, 32768] fp32 where partition p = h*64 + c (h = n-half,
  c = channel), streamed in [128, 4096] tiles (two [64, 4096] DMAs each;
  16 KiB contiguous rows amortize the ~300ns/descriptor SDMA overhead so
  a single HWDGE queue sustains ~300 GB/s instead of ~200 at 8 KiB rows).

  Split: hi = bf16(q) (DVE 2x copy), lo = bf16(q - hi) (fp32 TT, split
  DVE/GpSimd). hi+lo reconstructs q to ~2^-18; hi-only energy fails
  (rel 0.16) but hi/lo energy + hi-only phase-2 passes at ~3e-3.

  Phase 1 (energy): PE-transpose [128, 128] bf16 blocks of hi/lo, stage
  PPG pairs [Thi|Tlo] per PSUM group, copy to SBUF (ACT/DVE/GpSimd round
  robin), then bf16 grams accumulate acc[:,0:128] += T^T@Thi (diag-block
  energies) and acc[:,128:256] += T^T@Tlo (hi/lo cross term).
  E = (G00+G11) + (X00+X11) + (X00+X11)^T via matmul against the stacked
  double identity.

  Softmax: att = exp(rmin(E) - E) / rowsum (shift-invariant, matches the
  reference). S = blockdiag(M^T, M^T), M = gamma*att + I, cast bf16.

  Phase 2: out_slab = S_hi @ hi_slab (single bf16 matmul; the identity
  carries the hi residual; dropping the lo terms costs ~2e-3 rel).

  Schedule: reads on the Sync HWDGE ring, writes on the Scalar HWDGE
  ring (independent rings overlap read+write streams). Batch 1's
  load/split/phase-1 is interleaved tile-by-tile with batch 0's
  phase-2/stores so the write stream starts ~55us in and the PE never
  sits behind a write-paced head-of-line stall.
"""

import os

import numpy as np

import concourse.bass as bass
import concourse.tile as tile
from concourse import bacc, mybir

# Problem constants (hardcoded per harness contract).
B, C, H, W = 16, 64, 256, 256
N = H * W  # 65536
NCORES = 8
BPC = B // NCORES  # batches per core

# Tunables.
TILE_F = 4096  # free width of a q2 tile ([64, TILE_F] fp32 DMA rows)
CHUNK = 128  # n'-block width (covers both halves per transpose)
PPG = 4  # transpose pairs per PSUM staging group (1 bank)
SLAB = 512  # phase-2 moving width (one PSUM bank of fp32)


def build_cam_program(n=N, bpc=BPC, tile_f=TILE_F):
    """Build the single-core Bass program (same program runs on all cores)."""
    half = n // 2
    ntiles = half // tile_f
    fp32 = mybir.dt.float32
    bf16 = mybir.dt.bfloat16

    nc = bacc.Bacc("TRN2", target_bir_lowering=False, debug=False)
    x = nc.dram_tensor("x", [bpc, C, n], fp32, kind="ExternalInput").ap()
    gamma = nc.dram_tensor("gamma", [1], fp32, kind="ExternalInput").ap()
    # ident: [128, 64] stacked double identity (fp32) for half-sum matmuls.
    ident = nc.dram_tensor("ident", [128, 64], fp32, kind="ExternalInput").ap()
    # identb: [128, 128] identity (bf16) as moving operand of bf16 transposes.
    identb = nc.dram_tensor("identb", [128, 128], bf16, kind="ExternalInput").ap()
    out = nc.dram_tensor("out", [bpc, C, n], fp32, kind="ExternalOutput").ap()

    blocks_per_tile = tile_f // CHUNK
    groups_per_tile = blocks_per_tile // PPG
    slabs_per_tile = tile_f // SLAB
    nblocks = ntiles * blocks_per_tile  # per batch

    with tile.TileContext(nc) as tc:
        with (
            tc.tile_pool(name="qpool", bufs=3) as qpool,
            tc.tile_pool(name="hipool", bufs=ntiles + 3) as hipool,
            tc.tile_pool(name="lopool", bufs=3) as lopool,
            tc.tile_pool(name="tpool", bufs=6) as tpool,
            tc.tile_pool(name="opool", bufs=2) as opool,
            tc.tile_pool(name="spool", bufs=1) as spool,
            tc.tile_pool(name="single", bufs=1) as single,
            tc.tile_pool(name="tps", bufs=5, space="PSUM") as tps_pool,
            tc.tile_pool(name="eps", bufs=1, space="PSUM") as eps_pool,
            tc.tile_pool(name="ops", bufs=2, space="PSUM") as ops_pool,
        ):
            # Constants ride the Scalar (qAct) HWDGE ring, which is idle at
            # start; x loads start immediately on the Sync (qSP) ring.
            ident_sb = single.tile([128, 64], fp32)
            nc.scalar.dma_start(out=ident_sb, in_=ident)
            identb_sb = single.tile([128, 128], bf16)
            nc.scalar.dma_start(out=identb_sb, in_=identb)
            gamma_sb = single.tile([128, 1], fp32)
            nc.scalar.dma_start(out=gamma_sb, in_=gamma.to_broadcast((128, 1)))

            # Warmup transpose: absorbs the identb-DMA wait on PE so real
            # transposes carry a single wait (LDWEIGHTS allows one).
            warm = ops_pool.tile([128, 128], bf16, tag="ops", name="warm")
            nc.tensor.transpose(warm, identb_sb, identb_sb)

            hitiles = {}  # (b, t) -> hi tile

            def load_split(b, t, dma_eng=None):
                """DMA one [128, tile_f] fp32 tile and hi/lo split it.

                dma_eng picks the HWDGE ring: sync (qSP) or scalar (qAct).
                Per-queue DMA rate caps at ~224 GB/s (14 GB/s x 16 SDMA
                engines regardless of row size), so read-only phases
                alternate tiles across both rings.
                """
                eng = dma_eng or nc.sync
                qt = qpool.tile([128, tile_f], fp32)
                eng.dma_start(
                    out=qt[0:64, :], in_=x[b, :, t * tile_f : (t + 1) * tile_f]
                )
                eng.dma_start(
                    out=qt[64:128, :],
                    in_=x[b, :, half + t * tile_f : half + (t + 1) * tile_f],
                )
                hi = hipool.tile([128, tile_f], bf16)
                nc.vector.tensor_copy(out=hi, in_=qt)  # DVE 2x fp32 copy-cast
                lo = lopool.tile([128, tile_f], bf16)
                # GpSimd cannot touch PSUM, so it owns most of the subtract
                # (SBUF-only) while DVE takes a quarter; split so the first
                # transposes start before the whole tile is split.
                cut1 = (tile_f // 8) * 3
                cut2 = (tile_f // 8) * 6
                nc.gpsimd.tensor_tensor(
                    out=lo[:, 0:cut1],
                    in0=qt[:, 0:cut1],
                    in1=hi[:, 0:cut1],
                    op=mybir.AluOpType.subtract,
                )
                nc.gpsimd.tensor_tensor(
                    out=lo[:, cut1:cut2],
                    in0=qt[:, cut1:cut2],
                    in1=hi[:, cut1:cut2],
                    op=mybir.AluOpType.subtract,
                )
                nc.vector.tensor_tensor(
                    out=lo[:, cut2:],
                    in0=qt[:, cut2:],
                    in1=hi[:, cut2:],
                    op=mybir.AluOpType.subtract,
                )
                hitiles[(b, t)] = hi
                return lo

            def phase1_tile(b, t, lo, acc, gcnt):
                """Transpose + gram one tile into the batch accumulator."""
                hi = hitiles[(b, t)]
                # DVE is ~1.6x faster than ACT at PSUM-source copies; give
                # it the larger share.
                stage_eng = [
                    nc.vector,
                    nc.vector,
                    nc.scalar,
                    nc.vector,
                    nc.scalar,
                    nc.vector,
                    nc.vector,
                    nc.scalar,
                ]
                for g in range(groups_per_tile):
                    tps = tps_pool.tile([128, PPG * 256], bf16, tag="tps")
                    for i in range(PPG):
                        c = (g * PPG + i) * CHUNK
                        nc.tensor.transpose(
                            tps[:, i * 256 : i * 256 + 128],
                            hi[:, c : c + CHUNK],
                            identb_sb,
                        )
                        nc.tensor.transpose(
                            tps[:, i * 256 + 128 : (i + 1) * 256],
                            lo[:, c : c + CHUNK],
                            identb_sb,
                        )
                    tsb = tpool.tile([128, PPG * 256], bf16, tag="tsb")
                    eng = stage_eng[g % len(stage_eng)]
                    if eng is nc.scalar:
                        eng.copy(out=tsb, in_=tps)
                    else:
                        eng.tensor_copy(out=tsb, in_=tps)
                    for i in range(PPG):
                        nc.tensor.matmul(
                            acc,
                            lhsT=tsb[:, i * 256 : i * 256 + 128],
                            rhs=tsb[:, i * 256 : (i + 1) * 256],
                            start=(gcnt == 0),
                            stop=(gcnt == nblocks - 1),
                        )
                        gcnt += 1
                return gcnt

            def softmax_build_s(acc):
                """Combine energy halves, softmax, build the phase-2 stationary."""
                esb = spool.tile([128, 128], fp32)
                nc.vector.tensor_copy(out=esb[0:64, 0:64], in_=acc[0:64, 0:64])
                nc.vector.tensor_copy(
                    out=esb[64:128, 0:64], in_=acc[64:128, 64:128]
                )
                nc.vector.tensor_copy(
                    out=esb[0:64, 64:128], in_=acc[0:64, 128:192]
                )
                nc.vector.tensor_copy(
                    out=esb[64:128, 64:128], in_=acc[64:128, 192:256]
                )
                msum = ops_pool.tile([64, 128], fp32, tag="ops", name="msum")
                nc.tensor.matmul(
                    msum[:, 0:64],
                    lhsT=ident_sb,
                    rhs=esb[:, 0:64],
                    start=True,
                    stop=True,
                )
                nc.tensor.matmul(
                    msum[:, 64:128],
                    lhsT=ident_sb,
                    rhs=esb[:, 64:128],
                    start=True,
                    stop=True,
                )
                msb = spool.tile([64, 128], fp32)
                nc.vector.tensor_copy(out=msb, in_=msum)
                efull = spool.tile([64, 64], fp32)
                # E = E_hh + Xs + Xs^T
                xt = ops_pool.tile([64, 64], fp32, tag="ops", name="xt")
                nc.tensor.transpose(xt, msb[:, 64:128], ident_sb[0:64, :])
                nc.vector.tensor_add(efull, msb[:, 0:64], msb[:, 64:128])
                nc.vector.tensor_add(efull, efull, xt)

                # att = exp(rmin - E) / rowsum  (== softmax(rowmax(E)-E) rows)
                rmin = spool.tile([64, 1], fp32)
                nc.vector.tensor_reduce(
                    rmin, efull, axis=mybir.AxisListType.X, op=mybir.AluOpType.min
                )
                e2 = spool.tile([64, 128], fp32)
                nc.scalar.activation(
                    e2[:, 0:64],
                    efull,
                    mybir.ActivationFunctionType.Exp,
                    bias=rmin,
                    scale=-1.0,
                )
                ssum = spool.tile([64, 1], fp32)
                nc.vector.reduce_sum(ssum, e2[:, 0:64], axis=mybir.AxisListType.X)
                rsum = spool.tile([64, 1], fp32)
                nc.vector.reciprocal(rsum, ssum)
                att2 = spool.tile([64, 128], fp32)
                nc.vector.tensor_scalar_mul(att2[:, 0:64], e2[:, 0:64], rsum)
                nc.vector.tensor_copy(out=att2[:, 64:128], in_=att2[:, 0:64])
                # attT = [att^T ; att^T]
                atps = ops_pool.tile([128, 64], fp32, tag="ops", name="atps")
                nc.tensor.transpose(atps, att2, ident_sb[0:64, :])
                # S = blockdiag(M^T, M^T), M = gamma*att + I, cast bf16. The
                # identity carries the hi residual through phase 2.
                ssb = spool.tile([128, 128], fp32)
                nc.vector.memset(ssb, 0.0)
                nc.vector.tensor_scalar_mul(
                    ssb[0:64, 0:64], atps[0:64, :], gamma_sb[0:64]
                )
                nc.vector.tensor_scalar_mul(
                    ssb[64:128, 64:128], atps[64:128, :], gamma_sb[64:128]
                )
                nc.vector.tensor_add(
                    ssb[0:64, 0:64], ssb[0:64, 0:64], ident_sb[0:64, :]
                )
                nc.vector.tensor_add(
                    ssb[64:128, 64:128], ssb[64:128, 64:128], ident_sb[64:128, :]
                )
                s_hi = spool.tile([128, 128], bf16)
                nc.vector.tensor_copy(out=s_hi, in_=ssb)
                return s_hi

            def phase2_tile(b, t, s_hi, two_queues=False):
                """out tile = S_hi @ hi tile, copy out, store.

                two_queues: split the two half-stores across both HWDGE
                rings (tail only — sync is free once reads are done).
                """
                hi = hitiles.pop((b, t))
                osb = opool.tile([128, tile_f], fp32)
                copy_eng = [
                    nc.scalar,
                    nc.vector,
                    nc.scalar,
                    nc.scalar,
                    nc.vector,
                    nc.scalar,
                    nc.vector,
                    nc.scalar,
                ]
                for s in range(slabs_per_tile):
                    sl = slice(s * SLAB, (s + 1) * SLAB)
                    ops = ops_pool.tile([128, SLAB], fp32)
                    nc.tensor.matmul(
                        ops, lhsT=s_hi, rhs=hi[:, sl], start=True, stop=True
                    )
                    eng = copy_eng[s % len(copy_eng)]
                    if eng is nc.scalar:
                        eng.copy(out=osb[:, sl], in_=ops)
                    else:
                        eng.tensor_copy(out=osb[:, sl], in_=ops)
                eng2 = nc.sync if two_queues else nc.scalar
                nc.scalar.dma_start(
                    out=out[b, :, t * tile_f : (t + 1) * tile_f],
                    in_=osb[0:64, :],
                )
                eng2.dma_start(
                    out=out[b, :, half + t * tile_f : half + (t + 1) * tile_f],
                    in_=osb[64:128, :],
                )

            # ---- Batch 0: load (both HWDGE rings — no writes yet) + phase 1
            acc0 = eps_pool.tile([128, 256], fp32, tag="gacc")
            gcnt = 0
            lotiles0 = [
                load_split(0, t, dma_eng=(nc.sync if t % 2 == 0 else nc.scalar))
                for t in range(ntiles)
            ]
            for t in range(ntiles):
                gcnt = phase1_tile(0, t, lotiles0[t], acc0, gcnt)
            s_hi0 = softmax_build_s(acc0)

            # ---- Mixed: batch 1 load/phase-1 (qSP) interleaved with batch 0
            # phase-2/stores (qAct) — keeps both rings and the PE dense ----
            acc1 = eps_pool.tile([128, 256], fp32, tag="gacc")
            gcnt = 0
            for t in range(ntiles):
                lo = load_split(1, t, dma_eng=nc.sync)
                gcnt = phase1_tile(1, t, lo, acc1, gcnt)
                phase2_tile(0, t, s_hi0)
            s_hi1 = softmax_build_s(acc1)

            # ---- Tail: batch 1 phase 2, stores on both rings ----
            for t in range(ntiles):
                phase2_tile(1, t, s_hi1, two_queues=True)

    if not nc.is_finalized():
        nc.finalize()
    return nc


def _make_ident():
    ident = np.zeros((128, 64), np.float32)
    ident[np.arange(64), np.arange(64)] = 1.0
    ident[64 + np.arange(64), np.arange(64)] = 1.0
    return ident


def _make_identb():
    import ml_dtypes

    return np.eye(128, dtype=ml_dtypes.bfloat16)


def _setup_trace_hook():
    """Register the axon NTFF profiling hook (the image's antenv lacks the
    axon_hooks shim module; rebuild it and wire it to libaxon_pjrt.so)."""
    import sys
    import types

    import antenv

    if "antenv.axon_hooks" not in sys.modules:
        mod = types.ModuleType("antenv.axon_hooks")
        mod._hook = None

        def set_axon_ntff_profile_hook(h):
            mod._hook = h

        def get_axon_ntff_profile_hook():
            return mod._hook

        mod.set_axon_ntff_profile_hook = set_axon_ntff_profile_hook
        mod.get_axon_ntff_profile_hook = get_axon_ntff_profile_hook
        sys.modules["antenv.axon_hooks"] = mod
        antenv.axon_hooks = mod

    hooks = sys.modules["antenv.axon_hooks"]
    if hooks.get_axon_ntff_profile_hook() is None:
        from trn_agent_boot.trn_boot import _ntff_profile_via_ctypes

        hooks.set_axon_ntff_profile_hook(
            _ntff_profile_via_ctypes("/opt/axon/libaxon_pjrt.so")
        )

    # No S3 in this container: keep profile artifacts local.
    import concourse.bass_utils as bu

    bu.upload_artifacts = lambda tmpdir: tmpdir


def run(x, gamma, trace=False, tmpdir=None):
    """Run the SPMD kernel on 8 cores. Returns (out, exec_time_ns_or_None)."""
    from concourse.bass_utils import run_bass_kernel_spmd

    if trace:
        try:
            _setup_trace_hook()
        except Exception as e:  # tracing is best-effort; execution still works
            print("trace setup failed:", e)

    x = np.ascontiguousarray(np.asarray(x, dtype=np.float32))
    gamma = np.ascontiguousarray(np.asarray(gamma, dtype=np.float32))
    assert x.shape == (B, C, H, W), x.shape

    nc = build_cam_program()
    ident = _make_ident()
    identb = _make_identb()
    xr = x.reshape(NCORES, BPC, C, N)
    in_maps = [
        {
            "x": np.ascontiguousarray(xr[i]),
            "gamma": gamma,
            "ident": ident,
            "identb": identb,
        }
        for i in range(NCORES)
    ]
    res = run_bass_kernel_spmd(
        nc, in_maps, core_ids=list(range(NCORES)), trace=trace, tmpdir=tmpdir
    )
    outs = np.stack([np.asarray(res.results[i]["out"]) for i in range(NCORES)])
    y = outs.reshape(B, C, H, W).astype(np.float32)
    return y, res.exec_time_ns


def kernel(x, gamma):
    y, _ = run(x, gamma)
    return y


# revision 14
# speedup vs baseline: 1.1680x; 1.0664x over previous
"""CAM (channel attention) module kernel for Trainium2, SPMD over 8 NeuronCores.

Reference computation (per batch b):
    q = x[b].reshape(C, N)                  # C=64, N=H*W=65536
    energy = q @ q.T                        # [C, C]
    att = softmax(rowmax(energy) - energy)  # == softmax(-energy) rows
    out[b] = gamma * (att @ q) + x[b]

Sharding: data-parallel over batch, 2 batches per core, no cross-core comm.

Per-core design (v4):

  Layout: q2 [128, 32768] fp32 where partition p = h*64 + c (h = n-half,
  c = channel), streamed in [128, 2048] tiles (two [64, 2048] DMAs each).

  Numerics: hi = bf16(q), lo = bf16(q - hi). hi-only energy fails (rel
  0.16: bf16 E error ~2.5 vs row min-gaps ~0.06) but hi/lo energy +
  hi-only phase-2 passes at 3.3e-3 (gate 2e-2).

  Phase 1 (energy): PE-transpose [128, 128] bf16 blocks of hi/lo, stage
  4 pairs [Thi|Tlo] per PSUM bank, copy to SBUF (DVE/ACT alternate),
  then bf16 grams accumulate acc[:,0:128] += T^T@Thi (diag-block
  energies) and acc[:,128:256] += T^T@Tlo (hi/lo cross term).
  E = (G00+G11) + (X00+X11) + (X00+X11)^T via matmul against the
  stacked double identity.

  Softmax: att = exp(rmin(E) - E) / rowsum (shift-invariant, matches
  the reference). S = blockdiag(M^T, M^T), M = gamma*att + I, cast
  bf16; the identity carries the hi residual through phase 2.

  Phase 2: out_slab = S_hi @ hi_slab (single bf16 matmul per slab).

  DMA schedule (per-queue rate caps at ~224 GB/s = 16 SDMA engines x
  ~14 GB/s regardless of row size; concurrent queues share up to ~342):
    head:  b0 reads on qSP (sync)          -- scalar free for constants
    mixed: b1 reads on qSP, b0 stores qAct -- two dense streams
    tail:  b1 stores round-robin on qAct + qSP + qPool(SWDGE)
  Issue order is software-pipelined (loads stay PREFETCH tiles ahead;
  phase-1 of b1 interleaves phase-2 of b0 tile-by-tile) so no engine
  queue ever holds a long blocking wait in front of compute work.
"""

import numpy as np

import concourse.bass as bass
import concourse.tile as tile
from concourse import bacc, mybir

# Problem constants (hardcoded per harness contract).
B, C, H, W = 16, 64, 256, 256
N = H * W  # 65536
NCORES = 8
BPC = B // NCORES  # batches per core

# Tunables.
TILE_F = 2048  # free width of a q2 tile
CHUNK = 128  # n'-block width (covers both halves per transpose)
PPG = 4  # transpose pairs per PSUM staging group (1 bank)
SLAB = 512  # phase-2 moving width (one PSUM bank of fp32)
PREFETCH = 4  # tiles of read-ahead


def build_cam_program(n=N, bpc=BPC, tile_f=TILE_F):
    """Build the single-core Bass program (same program runs on all cores)."""
    half = n // 2
    ntiles = half // tile_f
    fp32 = mybir.dt.float32
    bf16 = mybir.dt.bfloat16

    nc = bacc.Bacc("TRN2", target_bir_lowering=False, debug=False)
    x = nc.dram_tensor("x", [bpc, C, n], fp32, kind="ExternalInput").ap()
    gamma = nc.dram_tensor("gamma", [1], fp32, kind="ExternalInput").ap()
    # ident: [128, 64] stacked double identity (fp32) for half-sum matmuls.
    ident = nc.dram_tensor("ident", [128, 64], fp32, kind="ExternalInput").ap()
    # identb: [128, 128] identity (bf16) as moving operand of bf16 transposes.
    identb = nc.dram_tensor("identb", [128, 128], bf16, kind="ExternalInput").ap()
    out = nc.dram_tensor("out", [bpc, C, n], fp32, kind="ExternalOutput").ap()

    blocks_per_tile = tile_f // CHUNK
    groups_per_tile = blocks_per_tile // PPG
    slabs_per_tile = tile_f // SLAB
    nblocks = ntiles * blocks_per_tile  # per batch

    with tile.TileContext(nc) as tc:
        with (
            tc.tile_pool(name="qpool", bufs=3) as qpool,
            tc.tile_pool(name="hipool", bufs=ntiles + 2) as hipool,
            tc.tile_pool(name="lopool", bufs=8) as lopool,
            tc.tile_pool(name="tpool", bufs=6) as tpool,
            tc.tile_pool(name="opool", bufs=4) as opool,
            tc.tile_pool(name="spool", bufs=1) as spool,
            tc.tile_pool(name="single", bufs=1) as single,
            tc.tile_pool(name="tps", bufs=5, space="PSUM") as tps_pool,
            tc.tile_pool(name="eps", bufs=1, space="PSUM") as eps_pool,
            tc.tile_pool(name="ops", bufs=2, space="PSUM") as ops_pool,
        ):
            # Constants ride the Scalar (qAct) ring, idle until stores start;
            # x loads start immediately on the Sync (qSP) ring.
            ident_sb = single.tile([128, 64], fp32)
            nc.scalar.dma_start(out=ident_sb, in_=ident)
            identb_sb = single.tile([128, 128], bf16)
            nc.scalar.dma_start(out=identb_sb, in_=identb)
            gamma_sb = single.tile([128, 1], fp32)
            nc.scalar.dma_start(out=gamma_sb, in_=gamma.to_broadcast((128, 1)))

            # Warmup transpose: absorbs the identb-DMA wait on PE so real
            # transposes carry a single wait (LDWEIGHTS allows one).
            warm = ops_pool.tile([128, 128], bf16, tag="ops", name="warm")
            nc.tensor.transpose(warm, identb_sb, identb_sb)

            hitiles = {}  # (b, t) -> hi tile

            def load_split(b, t):
                """DMA one [128, tile_f] fp32 tile (qSP) and hi/lo split it."""
                qt = qpool.tile([128, tile_f], fp32)
                nc.sync.dma_start(
                    out=qt[0:64, :], in_=x[b, :, t * tile_f : (t + 1) * tile_f]
                )
                nc.sync.dma_start(
                    out=qt[64:128, :],
                    in_=x[b, :, half + t * tile_f : half + (t + 1) * tile_f],
                )
                hi = hipool.tile([128, tile_f], bf16)
                nc.vector.tensor_copy(out=hi, in_=qt)  # DVE 2x fp32 copy-cast
                lo = lopool.tile([128, tile_f], bf16)
                # GpSimd owns the subtract (it cannot read PSUM, so the
                # PSUM-side copies stay on DVE/ACT); halves so the first
                # transposes start before the whole tile is split.
                hw = tile_f // 2
                nc.gpsimd.tensor_tensor(
                    out=lo[:, 0:hw],
                    in0=qt[:, 0:hw],
                    in1=hi[:, 0:hw],
                    op=mybir.AluOpType.subtract,
                )
                nc.gpsimd.tensor_tensor(
                    out=lo[:, hw:],
                    in0=qt[:, hw:],
                    in1=hi[:, hw:],
                    op=mybir.AluOpType.subtract,
                )
                hitiles[(b, t)] = hi
                return lo

            def phase1_tile(b, t, lo, acc, gcnt):
                """Transpose + gram one tile into the batch accumulator."""
                hi = hitiles[(b, t)]
                stage_eng = [nc.vector, nc.scalar]
                for g in range(groups_per_tile):
                    tps = tps_pool.tile([128, PPG * 256], bf16, tag="tps")
                    for i in range(PPG):
                        c = (g * PPG + i) * CHUNK
                        nc.tensor.transpose(
                            tps[:, i * 256 : i * 256 + 128],
                            hi[:, c : c + CHUNK],
                            identb_sb,
                        )
                        nc.tensor.transpose(
                            tps[:, i * 256 + 128 : (i + 1) * 256],
                            lo[:, c : c + CHUNK],
                            identb_sb,
                        )
                    tsb = tpool.tile([128, PPG * 256], bf16, tag="tsb")
                    eng = stage_eng[g % len(stage_eng)]
                    if eng is nc.scalar:
                        eng.copy(out=tsb, in_=tps)
                    else:
                        eng.tensor_copy(out=tsb, in_=tps)
                    for i in range(PPG):
                        nc.tensor.matmul(
                            acc,
                            lhsT=tsb[:, i * 256 : i * 256 + 128],
                            rhs=tsb[:, i * 256 : (i + 1) * 256],
                            start=(gcnt == 0),
                            stop=(gcnt == nblocks - 1),
                        )
                        gcnt += 1
                return gcnt

            def softmax_build_s(acc):
                """Combine energy halves, softmax, build the phase-2 stationary."""
                esb = spool.tile([128, 128], fp32)
                nc.vector.tensor_copy(out=esb[0:64, 0:64], in_=acc[0:64, 0:64])
                nc.vector.tensor_copy(
                    out=esb[64:128, 0:64], in_=acc[64:128, 64:128]
                )
                nc.vector.tensor_copy(
                    out=esb[0:64, 64:128], in_=acc[0:64, 128:192]
                )
                nc.vector.tensor_copy(
                    out=esb[64:128, 64:128], in_=acc[64:128, 192:256]
                )
                msum = ops_pool.tile([64, 128], fp32, tag="ops", name="msum")
                nc.tensor.matmul(
                    msum[:, 0:64],
                    lhsT=ident_sb,
                    rhs=esb[:, 0:64],
                    start=True,
                    stop=True,
                )
                nc.tensor.matmul(
                    msum[:, 64:128],
                    lhsT=ident_sb,
                    rhs=esb[:, 64:128],
                    start=True,
                    stop=True,
                )
                msb = spool.tile([64, 128], fp32)
                nc.vector.tensor_copy(out=msb, in_=msum)
                efull = spool.tile([64, 64], fp32)
                # E = E_hh + Xs + Xs^T
                xt = ops_pool.tile([64, 64], fp32, tag="ops", name="xt")
                nc.tensor.transpose(xt, msb[:, 64:128], ident_sb[0:64, :])
                nc.vector.tensor_add(efull, msb[:, 0:64], msb[:, 64:128])
                nc.vector.tensor_add(efull, efull, xt)

                # att = exp(rmin - E) / rowsum  (== softmax(rowmax(E)-E) rows)
                rmin = spool.tile([64, 1], fp32)
                nc.vector.tensor_reduce(
                    rmin, efull, axis=mybir.AxisListType.X, op=mybir.AluOpType.min
                )
                e2 = spool.tile([64, 128], fp32)
                nc.scalar.activation(
                    e2[:, 0:64],
                    efull,
                    mybir.ActivationFunctionType.Exp,
                    bias=rmin,
                    scale=-1.0,
                )
                ssum = spool.tile([64, 1], fp32)
                nc.vector.reduce_sum(ssum, e2[:, 0:64], axis=mybir.AxisListType.X)
                rsum = spool.tile([64, 1], fp32)
                nc.vector.reciprocal(rsum, ssum)
                att2 = spool.tile([64, 128], fp32)
                nc.vector.tensor_scalar_mul(att2[:, 0:64], e2[:, 0:64], rsum)
                nc.vector.tensor_copy(out=att2[:, 64:128], in_=att2[:, 0:64])
                # attT = [att^T ; att^T]
                atps = ops_pool.tile([128, 64], fp32, tag="ops", name="atps")
                nc.tensor.transpose(atps, att2, ident_sb[0:64, :])
                # S = blockdiag(M^T, M^T), M = gamma*att + I, cast bf16.
                ssb = spool.tile([128, 128], fp32)
                nc.vector.memset(ssb, 0.0)
                nc.vector.tensor_scalar_mul(
                    ssb[0:64, 0:64], atps[0:64, :], gamma_sb[0:64]
                )
                nc.vector.tensor_scalar_mul(
                    ssb[64:128, 64:128], atps[64:128, :], gamma_sb[64:128]
                )
                nc.vector.tensor_add(
                    ssb[0:64, 0:64], ssb[0:64, 0:64], ident_sb[0:64, :]
                )
                nc.vector.tensor_add(
                    ssb[64:128, 64:128], ssb[64:128, 64:128], ident_sb[64:128, :]
                )
                s_hi = spool.tile([128, 128], bf16)
                nc.vector.tensor_copy(out=s_hi, in_=ssb)
                return s_hi

            def phase2_tile(b, t, s_hi, store_engs=(None, None)):
                """out tile = S_hi @ hi tile, copy out, store."""
                hi = hitiles.pop((b, t))
                osb = opool.tile([128, tile_f], fp32)
                copy_eng = [nc.scalar, nc.vector]
                for s in range(slabs_per_tile):
                    sl = slice(s * SLAB, (s + 1) * SLAB)
                    ops = ops_pool.tile([128, SLAB], fp32)
                    nc.tensor.matmul(
                        ops, lhsT=s_hi, rhs=hi[:, sl], start=True, stop=True
                    )
                    eng = copy_eng[s % len(copy_eng)]
                    if eng is nc.scalar:
                        eng.copy(out=osb[:, sl], in_=ops)
                    else:
                        eng.tensor_copy(out=osb[:, sl], in_=ops)
                e0 = store_engs[0] or nc.scalar
                e1 = store_engs[1] or nc.scalar
                e0.dma_start(
                    out=out[b, :, t * tile_f : (t + 1) * tile_f],
                    in_=osb[0:64, :],
                )
                e1.dma_start(
                    out=out[b, :, half + t * tile_f : half + (t + 1) * tile_f],
                    in_=osb[64:128, :],
                )

            # ---- Head: batch 0 load (qSP) + phase 1, loads PREFETCH ahead --
            acc0 = eps_pool.tile([128, 256], fp32, tag="gacc")
            gcnt = 0
            los = {}
            for t in range(PREFETCH):
                los[(0, t)] = load_split(0, t)
            for t in range(ntiles):
                if t + PREFETCH < ntiles:
                    los[(0, t + PREFETCH)] = load_split(0, t + PREFETCH)
                gcnt = phase1_tile(0, t, los.pop((0, t)), acc0, gcnt)
            s_hi0 = softmax_build_s(acc0)

            # ---- Mixed: batch 1 load/phase-1 (qSP) interleaved with batch 0
            # phase-2/stores (qAct) ----
            acc1 = eps_pool.tile([128, 256], fp32, tag="gacc")
            gcnt = 0
            for t in range(PREFETCH):
                los[(1, t)] = load_split(1, t)
            for t in range(ntiles):
                if t + PREFETCH < ntiles:
                    los[(1, t + PREFETCH)] = load_split(1, t + PREFETCH)
                gcnt = phase1_tile(1, t, los.pop((1, t)), acc1, gcnt)
                phase2_tile(0, t, s_hi0)
            s_hi1 = softmax_build_s(acc1)

            # ---- Tail: batch 1 phase 2, stores across qAct/qSP/qPool ----
            tail_engs = [
                (nc.scalar, nc.sync),
                (nc.gpsimd, nc.scalar),
                (nc.sync, nc.gpsimd),
            ]
            for t in range(ntiles):
                phase2_tile(1, t, s_hi1, store_engs=tail_engs[t % 3])

    if not nc.is_finalized():
        nc.finalize()
    return nc


def _make_ident():
    ident = np.zeros((128, 64), np.float32)
    ident[np.arange(64), np.arange(64)] = 1.0
    ident[64 + np.arange(64), np.arange(64)] = 1.0
    return ident


def _make_identb():
    import ml_dtypes

    return np.eye(128, dtype=ml_dtypes.bfloat16)


def _setup_trace_hook():
    """Register the axon NTFF profiling hook (the image's antenv lacks the
    axon_hooks shim module; rebuild it and wire it to libaxon_pjrt.so)."""
    import sys
    import types

    import antenv

    if "antenv.axon_hooks" not in sys.modules:
        mod = types.ModuleType("antenv.axon_hooks")
        mod._hook = None

        def set_axon_ntff_profile_hook(h):
            mod._hook = h

        def get_axon_ntff_profile_hook():
            return mod._hook

        mod.set_axon_ntff_profile_hook = set_axon_ntff_profile_hook
        mod.get_axon_ntff_profile_hook = get_axon_ntff_profile_hook
        sys.modules["antenv.axon_hooks"] = mod
        antenv.axon_hooks = mod

    hooks = sys.modules["antenv.axon_hooks"]
    if hooks.get_axon_ntff_profile_hook() is None:
        from trn_agent_boot.trn_boot import _ntff_profile_via_ctypes

        hooks.set_axon_ntff_profile_hook(
            _ntff_profile_via_ctypes("/opt/axon/libaxon_pjrt.so")
        )

    # No S3 in this container: keep profile artifacts local.
    import concourse.bass_utils as bu

    bu.upload_artifacts = lambda tmpdir: tmpdir


def run(x, gamma, trace=False, tmpdir=None):
    """Run the SPMD kernel on 8 cores. Returns (out, exec_time_ns_or_None)."""
    from concourse.bass_utils import run_bass_kernel_spmd

    if trace:
        try:
            _setup_trace_hook()
        except Exception as e:  # tracing is best-effort; execution still works
            print("trace setup failed:", e)

    x = np.ascontiguousarray(np.asarray(x, dtype=np.float32))
    gamma = np.ascontiguousarray(np.asarray(gamma, dtype=np.float32))
    assert x.shape == (B, C, H, W), x.shape

    nc = build_cam_program()
    ident = _make_ident()
    identb = _make_identb()
    xr = x.reshape(NCORES, BPC, C, N)
    in_maps = [
        {
            "x": np.ascontiguousarray(xr[i]),
            "gamma": gamma,
            "ident": ident,
            "identb": identb,
        }
        for i in range(NCORES)
    ]
    res = run_bass_kernel_spmd(
        nc, in_maps, core_ids=list(range(NCORES)), trace=trace, tmpdir=tmpdir
    )
    outs = np.stack([np.asarray(res.results[i]["out"]) for i in range(NCORES)])
    y = outs.reshape(B, C, H, W).astype(np.float32)
    return y, res.exec_time_ns


def kernel(x, gamma):
    y, _ = run(x, gamma)
    return y
